# revision 20
# baseline (speedup 1.0000x reference)
"""Causal multi-head attention block (B=2, T=2048, C=1024, H=16) on 8 TRN2
NeuronCores.

Sharding (Megatron-style): core = (b, hg) with b in {0,1} data-parallel over
batch and hg in {0..3} tensor-parallel over head groups (4 heads each).
Each core computes qkv for its 768 attn_w columns, attention for its 4 heads,
and a partial output projection; the host sums the 4 partials per batch.

Numerics: matmul operands are fp16 (full PE rate, 10-bit mantissa; PSUM
accumulates fp32); softmax is the unstable variant (logits are O(10), exp
cannot overflow); the k-bias is dropped (softmax row-shift invariant) and the
v-bias is constant-folded into an effective output-projection bias on the
host. Softmax reciprocals use the ~18-bit DVE fast approximation.

Attention layout: S^T = K Q^T is computed per head-pair with the two heads
row-packed in the PE array (contraction dim 64 each); softmax reductions then
live on the free axis via a ones-column appended to V (PV yields out^T with
the denominators in row 64). The two heads of a pair accumulate in one PSUM
tile packed along the free axis.
"""

import numpy as np

B, T, C = 2, 2048, 1024
H, HD = 16, 64
P = 128
TT = T // P      # 16 row tiles
NI = T // 512    # 4 query blocks of 512
CT = C // P      # 8 contraction tiles
SCALE = HD ** -0.5

_NC_CACHE = {}


def _build_nc(repeats=1):
    import concourse.tile as tile
    from concourse import bacc, mybir
    from concourse.bass import ds, ts

    f32 = mybir.dt.float32
    f16 = mybir.dt.float16
    AF = mybir.ActivationFunctionType

    nc = bacc.Bacc("TRN2", target_bir_lowering=False, debug=False)

    xT_d = nc.declare_dram_parameter("xT", [C, T], f16, isOutput=False)
    wqk_d = nc.declare_dram_parameter("wqk", [C, 512], f16, isOutput=False)
    wv_d = nc.declare_dram_parameter("wv", [C, 256], f16, isOutput=False)
    bq_d = nc.declare_dram_parameter("bq", [256], f32, isOutput=False)
    pw_d = nc.declare_dram_parameter("pw", [256, C], f16, isOutput=False)
    y_d = nc.declare_dram_parameter("y", [T, C], f16, isOutput=True)

    with (
        tile.TileContext(nc) as tc,
        tc.tile_pool(name="const", bufs=1) as constp,
        tc.tile_pool(name="xw", bufs=1) as xwp,
        tc.tile_pool(name="acts", bufs=1) as actsp,
        tc.tile_pool(name="pt", bufs=4) as ptp,
        tc.tile_pool(name="ac2", bufs=4) as ac2p,
        tc.tile_pool(name="rec", bufs=8) as recp,
        tc.tile_pool(name="rb", bufs=4) as rbp,
        tc.tile_pool(name="ysb", bufs=4) as ysbp,
        tc.tile_pool(name="qkps", bufs=2, space="PSUM") as qkps,
        tc.tile_pool(name="pvps", bufs=2, space="PSUM") as pvps,
    ):
        # ---------- small constants first ----------
        bq_sb = constp.tile([P, 2], f32)
        nc.sync.dma_start(bq_sb[:], bq_d.rearrange("(o p) -> p o", p=P))

        # ---------- big loads, most-needed-first ----------
        xT_sb = xwp.tile([P, CT, T], f16)
        xTr = xT_d.rearrange("(ct p) t -> p ct t", p=P)
        wqk_sb = xwp.tile([P, CT, 512], f16)
        wqkr = wqk_d.rearrange("(ct p) j -> p ct j", p=P)
        wv_sb = xwp.tile([P, CT, 256], f16)
        wvr = wv_d.rearrange("(ct p) j -> p ct j", p=P)
        for c in range(CT):
            nc.sync.dma_start(wqk_sb[:, c, :], wqkr[:, c, :])
            nc.sync.dma_start(xT_sb[:, c, 0:1024], xTr[:, c, 0:1024])
            nc.sync.dma_start(xT_sb[:, c, 1024:2048], xTr[:, c, 1024:2048])
        for c in range(CT):
            nc.sync.dma_start(wv_sb[:, c, :], wvr[:, c, :])
        pw_sb = constp.tile([P, 2, C], f16)
        nc.sync.dma_start(pw_sb[:], pw_d.rearrange("(k p) n -> p k n", p=P))

        for _rep in range(repeats):
            # ---------- qkv^T projection ----------
            # qkT layout: [128, 4, T]; jt 0,1 = k^T head-pairs 0,1; jt 2,3 = q^T.
            # Within a jt tile, partitions 0-63 = even head of the pair, 64-127 odd.
            qkT = actsp.tile([P, 4, T], f16, tag="qkT", name=f"qkT{_rep}")
            v_all = actsp.tile([P, TT, 4, 65], f16, tag="v_all", name=f"v_all{_rep}")

            def emit_qk_proj(jt):
                for tp in range(2):
                    ps = qkps.tile([P, 2, 512], f32, tag="qk", name=f"qkp{_rep}{jt}{tp}")
                    for c in range(CT):
                        for s in range(2):
                            nc.tensor.matmul(
                                ps[:, s, :],
                                wqk_sb[:, c, ts(jt, P)],
                                xT_sb[:, c, ds(1024 * tp + 512 * s, 512)],
                                start=(c == 0),
                                stop=(c == CT - 1),
                            )
                    out = qkT[:, jt, ds(1024 * tp, 1024)].rearrange(
                        "p (s x) -> p s x", s=2
                    )
                    if jt >= 2:
                        nc.vector.tensor_scalar(
                            out,
                            ps[:],
                            scalar1=bq_sb[:, jt - 2 : jt - 1],
                            scalar2=None,
                            op0=mybir.AluOpType.add,
                        )
                    else:
                        nc.vector.tensor_copy(out, ps[:])

            def emit_v_proj(tt):
                # v_all[p, tt, l, d]: t = 128*tt + p, head l, d 0-63; d=64 is ones.
                psv = qkps.tile([P, 2, 512], f32, tag="qk", name=f"vp{_rep}{tt}")
                for c in range(CT):
                    nc.tensor.matmul(
                        psv[:, 0, 0:256],
                        xT_sb[:, c, ts(tt, P)],
                        wv_sb[:, c, :],
                        start=(c == 0),
                        stop=(c == CT - 1),
                    )
                nc.vector.tensor_copy(
                    v_all[:, tt, :, 0:64],
                    psv[:, 0, 0:256].rearrange("p (l d) -> p l d", l=4),
                )
                nc.vector.tensor_scalar(
                    v_all[:, tt, :, 64:65],
                    psv[:, 0, 0:4].rearrange("p (l d) -> p l d", l=4),
                    scalar1=0.0,
                    scalar2=1.0,
                    op0=mybir.AluOpType.mult,
                    op1=mybir.AluOpType.add,
                )

            emit_qk_proj(0)  # k^T pair 0
            emit_qk_proj(2)  # q^T pair 0
            for tt in range(TT):
                emit_v_proj(tt)
            emit_qk_proj(1)  # k^T pair 1
            emit_qk_proj(3)  # q^T pair 1

            # ---------- output projection (psum slots shared with S tiles) ----------
            def emit_y(tt):
                psy = qkps.tile([P, 2, 512], f32, tag="qk", name=f"y{_rep}{tt}")
                for n in range(2):
                    for k in range(2):
                        nc.tensor.matmul(
                            psy[:, n, :],
                            att[:, k, ts(tt, P)],
                            pw_sb[:, k, ds(512 * n, 512)],
                            start=(k == 0),
                            stop=(k == 1),
                        )
                y_sb = ysbp.tile([P, 1024], f16, tag="ysb", name=f"ys{_rep}{tt}")
                nc.vector.tensor_copy(y_sb[:, 0:512], psy[:, 0, :])
                nc.vector.tensor_copy(y_sb[:, 512:1024], psy[:, 1, :])
                nc.sync.dma_start(y_d[ts(tt, P), :], y_sb[:])

            # ---------- attention ----------
            # S^T tiles: [t_k partitions, t_q free]; one exp per j0 covers both
            # heads of the pair; PV contracts j=t_k with v as lhsT, producing
            # out^T [65, t_q] per head (row 64 = softmax denominators). Both
            # heads of a pair accumulate into one [P, 2, 512] PSUM tile.
            att = actsp.tile([P, 2, T], f16, tag="att", name=f"att{_rep}")

            def qk_exp_step(hp, i0, j0):
                kT = qkT[:, hp, :]
                qT = qkT[:, 2 + hp, :]
                c0 = P * j0 - 512 * i0
                c0p = max(0, c0)
                w = 512 - c0p
                psS = qkps.tile(
                    [P, 2, 512], f32, tag="qk", name=f"s{_rep}{hp}{i0}{j0}"
                )
                for h01 in range(2):
                    nc.tensor.matmul(
                        psS[:, h01, ds(c0p, w)],
                        kT[64 * h01 : 64 * h01 + 64, ts(j0, P)],
                        qT[64 * h01 : 64 * h01 + 64, ds(512 * i0 + c0p, w)],
                        start=True,
                        stop=True,
                    )
                pt = ptp.tile(
                    [P, 2, 512], f16, tag="pt", name=f"pt{_rep}{hp}{i0}{j0}"
                )
                nc.scalar.activation(
                    pt[:, :, ds(c0p, w)],
                    psS[:, :, ds(c0p, w)],
                    AF.Exp,
                    scale=SCALE,
                )
                if c0 >= 0:
                    for h01 in range(2):
                        nc.gpsimd.affine_select(
                            out=pt[:, h01, ds(c0, P)],
                            in_=pt[:, h01, ds(c0, P)],
                            compare_op=mybir.AluOpType.is_ge,
                            fill=0.0,
                            base=0,
                            pattern=[[1, P]],
                            channel_multiplier=-1,
                        )
                return pt

            def pv_step(hp, i0, j0, nj, acc, pt):
                cp = max(0, P * j0 - 512 * i0)
                wp = 512 - cp
                for h01 in range(2):
                    nc.tensor.matmul(
                        acc[0:65, h01, ds(cp, wp)],
                        v_all[:, j0, 2 * hp + h01, :],
                        pt[:, h01, ds(cp, wp)],
                        start=(j0 == 0),
                        stop=(j0 == nj - 1),
                    )

            def drain_block(hp, i0, acc):
                # Copy numerators out of PSUM (freeing the accumulator) and
                # compute fast reciprocals of the denominator rows.
                ac2 = ac2p.tile([P, 512], f32, tag="ac2", name=f"ac{_rep}{hp}{i0}")
                recs = []
                for h01 in range(2):
                    nc.vector.tensor_copy(
                        ac2[64 * h01 : 64 * h01 + 64, :], acc[0:64, h01, :]
                    )
                    dn = recp.tile(
                        [1, 512], f32, tag="dn", name=f"dn{_rep}{hp}{i0}{h01}"
                    )
                    nc.vector.tensor_copy(dn[:], acc[64:65, h01, :])
                    rec = recp.tile(
                        [1, 512], f32, tag="rec", name=f"rc{_rep}{hp}{i0}{h01}"
                    )
                    nc.vector.reciprocal_approx_fast(rec[:], dn[:])
                    recs.append(rec)
                return ac2, recs

            def normalize_i0(i0, drained):
                # att[64*h01 .. , hp, block] = numerators * (1/denominator)
                for hp in range(2):
                    ac2, recs = drained[hp]
                    for h01 in range(2):
                        rb = rbp.tile(
                            [P, 512], f32, tag="rb", name=f"rb{_rep}{hp}{i0}{h01}"
                        )
                        nc.gpsimd.partition_broadcast(rb[:], recs[h01][:])
                        sl = slice(64 * h01, 64 * h01 + 64)
                        nc.vector.tensor_mul(
                            att[sl, hp, ds(512 * i0, 512)],
                            ac2[sl, :],
                            rb[sl, :],
                        )

            steps = []
            for i0 in range(NI):
                for hp in range(2):
                    nj = 4 * i0 + 4
                    for j0 in range(nj):
                        steps.append((hp, i0, j0, nj))
            LAG = 3
            accs_map = {}
            pts_map = {}
            drained = {}
            y_after = {}
            for p in range(len(steps) + LAG):
                if p < len(steps):
                    hp, i0, j0, nj = steps[p]
                    if j0 == 0:
                        accs_map[(hp, i0)] = pvps.tile(
                            [P, 2, 512], f32, tag="pv", name=f"acc{_rep}{hp}{i0}"
                        )
                    pts_map[p] = qk_exp_step(hp, i0, j0)
                if p >= LAG:
                    hp, i0, j0, nj = steps[p - LAG]
                    pv_step(hp, i0, j0, nj, accs_map[(hp, i0)], pts_map.pop(p - LAG))
                    if j0 == nj - 1:
                        drained[(hp, i0)] = drain_block(hp, i0, accs_map.pop((hp, i0)))
                        if hp == 1:
                            normalize_i0(i0, (drained.pop((0, i0)), drained.pop((1, i0))))
                            y_after[p + 4] = i0
                if p in y_after:
                    i0y = y_after.pop(p)
                    for tt in range(4 * i0y, 4 * i0y + 4):
                        emit_y(tt)
            for i0y in sorted(y_after.values()):
                for tt in range(4 * i0y, 4 * i0y + 4):
                    emit_y(tt)

    nc.compile()
    return nc


def _get_nc(repeats=1):
    key = ("nc", repeats)
    if key not in _NC_CACHE:
        _NC_CACHE[key] = _build_nc(repeats)
    return _NC_CACHE[key]


def _make_in_maps(x, attn_w, attn_b, proj_w, proj_b):
    _make_in_maps.beff = {}
    in_maps = []
    for core in range(8):
        b, hg = core // 4, core % 4
        cs = 256 * hg
        k_cols = attn_w[:, cs : cs + 256]
        q_cols = attn_w[:, 1024 + cs : 1024 + cs + 256]
        v_cols = attn_w[:, 2048 + cs : 2048 + cs + 256]
        b_q = attn_b[1024 + cs : 1024 + cs + 256]
        b_v = attn_b[2048 + cs : 2048 + cs + 256]
        pw = proj_w[cs : cs + 256, :]
        beff = (b_v.astype(np.float64) @ pw.astype(np.float64)).astype(np.float32)
        if hg == 0:
            beff = beff + proj_b
        _make_in_maps.beff[core] = beff
        in_maps.append(
            {
                "xT": np.ascontiguousarray(x[b].T.astype(np.float16)),
                "wqk": np.ascontiguousarray(
                    np.concatenate([k_cols, q_cols], axis=1).astype(np.float16)
                ),
                "wv": np.ascontiguousarray(v_cols.astype(np.float16)),
                "bq": np.ascontiguousarray(b_q),
                "pw": np.ascontiguousarray(pw.astype(np.float16)),
            }
        )
    return in_maps


def kernel(x, attn_w, attn_b, proj_w, proj_b, _spmd_kwargs=None):
    from concourse.bass_utils import run_bass_kernel_spmd

    x = np.asarray(x, dtype=np.float32)
    attn_w = np.asarray(attn_w, dtype=np.float32)
    attn_b = np.asarray(attn_b, dtype=np.float32)
    proj_w = np.asarray(proj_w, dtype=np.float32)
    proj_b = np.asarray(proj_b, dtype=np.float32)

    nc = _get_nc((_spmd_kwargs or {}).pop("repeats", 1) if _spmd_kwargs else 1)
    in_maps = _make_in_maps(x, attn_w, attn_b, proj_w, proj_b)
    res = run_bass_kernel_spmd(
        nc, in_maps, core_ids=list(range(8)), **(_spmd_kwargs or {})
    )
    out = np.zeros((B, T, C), dtype=np.float32)
    for core in range(8):
        out[core // 4] += res.results[core]["y"].astype(np.float32)
    for core in range(8):
        out[core // 4] += _make_in_maps.beff[core][None, :]
    if _spmd_kwargs:
        kernel.last_results = res
    return out


# revision 21
# speedup vs baseline: 1.0257x; 1.0257x over previous
"""Causal multi-head attention block (B=2, T=2048, C=1024, H=16) on 8 TRN2
NeuronCores.

Sharding (Megatron-style): core = (b, hg) with b in {0,1} data-parallel over
batch and hg in {0..3} tensor-parallel over head groups (4 heads each).
Each core computes qkv for its 768 attn_w columns, attention for its 4 heads,
and a partial output projection; the host sums the 4 partials per batch.

Numerics: matmul operands are fp16 (full PE rate, 10-bit mantissa; PSUM
accumulates fp32); softmax is the unstable variant (logits are O(10), exp
cannot overflow); the k-bias is dropped (softmax row-shift invariant) and the
v-bias is constant-folded into an effective output-projection bias on the
host. Softmax reciprocals use the ~18-bit DVE fast approximation.

Attention layout: S^T = K Q^T is computed per head-pair with the two heads
row-packed in the PE array (contraction dim 64 each); softmax reductions then
live on the free axis via a ones-column appended to V (PV yields out^T with
the denominators in row 64). The two heads of a pair accumulate in one PSUM
tile packed along the free axis.
"""

import numpy as np

B, T, C = 2, 2048, 1024
H, HD = 16, 64
P = 128
TT = T // P      # 16 row tiles
NI = T // 512    # 4 query blocks of 512
CT = C // P      # 8 contraction tiles
SCALE = HD ** -0.5

_NC_CACHE = {}


def _build_nc(repeats=1):
    import concourse.tile as tile
    from concourse import bacc, mybir
    from concourse.bass import ds, ts

    f32 = mybir.dt.float32
    f16 = mybir.dt.float16
    AF = mybir.ActivationFunctionType

    nc = bacc.Bacc("TRN2", target_bir_lowering=False, debug=False)

    xT_d = nc.declare_dram_parameter("xT", [C, T], f16, isOutput=False)
    wqk_d = nc.declare_dram_parameter("wqk", [C, 512], f16, isOutput=False)
    wv_d = nc.declare_dram_parameter("wv", [C, 256], f16, isOutput=False)
    bq_d = nc.declare_dram_parameter("bq", [256], f32, isOutput=False)
    pw_d = nc.declare_dram_parameter("pw", [256, C], f16, isOutput=False)
    y_d = nc.declare_dram_parameter("y", [T, C], f16, isOutput=True)

    with (
        tile.TileContext(nc) as tc,
        tc.tile_pool(name="const", bufs=1) as constp,
        tc.tile_pool(name="xw", bufs=1) as xwp,
        tc.tile_pool(name="acts", bufs=1) as actsp,
        tc.tile_pool(name="pt", bufs=4) as ptp,
        tc.tile_pool(name="ac2", bufs=3) as ac2p,
        tc.tile_pool(name="rec", bufs=6) as recp,
        tc.tile_pool(name="rb", bufs=3) as rbp,
        tc.tile_pool(name="ysb", bufs=4) as ysbp,
        tc.tile_pool(name="qkps", bufs=2, space="PSUM") as qkps,
        tc.tile_pool(name="pvps", bufs=2, space="PSUM") as pvps,
    ):
        # ---------- small constants first ----------
        bq_sb = constp.tile([P, 2], f32)
        nc.sync.dma_start(bq_sb[:], bq_d.rearrange("(o p) -> p o", p=P))

        # ---------- big loads, most-needed-first ----------
        xT_sb = xwp.tile([P, CT, T], f16)
        xTr = xT_d.rearrange("(ct p) t -> p ct t", p=P)
        wqk_sb = xwp.tile([P, CT, 512], f16)
        wqkr = wqk_d.rearrange("(ct p) j -> p ct j", p=P)
        wv_sb = xwp.tile([P, CT, 256], f16)
        wvr = wv_d.rearrange("(ct p) j -> p ct j", p=P)
        for c in range(CT):
            nc.sync.dma_start(wqk_sb[:, c, :], wqkr[:, c, :])
            nc.sync.dma_start(xT_sb[:, c, 0:1024], xTr[:, c, 0:1024])
            nc.sync.dma_start(xT_sb[:, c, 1024:2048], xTr[:, c, 1024:2048])
        for c in range(CT):
            nc.sync.dma_start(wv_sb[:, c, :], wvr[:, c, :])
        pw_sb = constp.tile([P, 2, C], f16)
        nc.sync.dma_start(pw_sb[:], pw_d.rearrange("(k p) n -> p k n", p=P))

        for _rep in range(repeats):
            # ---------- qkv^T projection ----------
            # qkT layout: [128, 4, T]; jt 0,1 = k^T head-pairs 0,1; jt 2,3 = q^T.
            # Within a jt tile, partitions 0-63 = even head of the pair, 64-127 odd.
            qkT = actsp.tile([P, 4, T], f16, tag="qkT", name=f"qkT{_rep}")
            v_all = actsp.tile([P, TT, 4, 65], f16, tag="v_all", name=f"v_all{_rep}")

            def emit_qk_proj(jt):
                for tp in range(2):
                    ps = qkps.tile([P, 2, 512], f32, tag="qk", name=f"qkp{_rep}{jt}{tp}")
                    for c in range(CT):
                        for s in range(2):
                            nc.tensor.matmul(
                                ps[:, s, :],
                                wqk_sb[:, c, ts(jt, P)],
                                xT_sb[:, c, ds(1024 * tp + 512 * s, 512)],
                                start=(c == 0),
                                stop=(c == CT - 1),
                            )
                    out = qkT[:, jt, ds(1024 * tp, 1024)].rearrange(
                        "p (s x) -> p s x", s=2
                    )
                    if jt >= 2:
                        nc.vector.tensor_scalar(
                            out,
                            ps[:],
                            scalar1=bq_sb[:, jt - 2 : jt - 1],
                            scalar2=None,
                            op0=mybir.AluOpType.add,
                        )
                    else:
                        nc.vector.tensor_copy(out, ps[:])

            def emit_v_proj(tt):
                # v_all[p, tt, l, d]: t = 128*tt + p, head l, d 0-63; d=64 is ones.
                psv = qkps.tile([P, 2, 512], f32, tag="qk", name=f"vp{_rep}{tt}")
                for c in range(CT):
                    nc.tensor.matmul(
                        psv[:, 0, 0:256],
                        xT_sb[:, c, ts(tt, P)],
                        wv_sb[:, c, :],
                        start=(c == 0),
                        stop=(c == CT - 1),
                    )
                nc.vector.tensor_copy(
                    v_all[:, tt, :, 0:64],
                    psv[:, 0, 0:256].rearrange("p (l d) -> p l d", l=4),
                )
                nc.vector.tensor_scalar(
                    v_all[:, tt, :, 64:65],
                    psv[:, 0, 0:4].rearrange("p (l d) -> p l d", l=4),
                    scalar1=0.0,
                    scalar2=1.0,
                    op0=mybir.AluOpType.mult,
                    op1=mybir.AluOpType.add,
                )

            emit_qk_proj(0)  # k^T pair 0
            emit_qk_proj(2)  # q^T pair 0
            for tt in range(TT):
                emit_v_proj(tt)
            emit_qk_proj(1)  # k^T pair 1
            emit_qk_proj(3)  # q^T pair 1

            # ---------- output projection (psum slots shared with S tiles) ----------
            def emit_y(tt):
                psy = qkps.tile([P, 2, 512], f32, tag="qk", name=f"y{_rep}{tt}")
                for n in range(2):
                    for k in range(2):
                        nc.tensor.matmul(
                            psy[:, n, :],
                            att[:, k, ts(tt, P)],
                            pw_sb[:, k, ds(512 * n, 512)],
                            start=(k == 0),
                            stop=(k == 1),
                        )
                y_sb = ysbp.tile([P, 1024], f16, tag="ysb", name=f"ys{_rep}{tt}")
                nc.vector.tensor_copy(y_sb[:, 0:512], psy[:, 0, :])
                nc.vector.tensor_copy(y_sb[:, 512:1024], psy[:, 1, :])
                nc.sync.dma_start(y_d[ts(tt, P), :], y_sb[:])

            # ---------- attention ----------
            # S^T tiles: [t_k partitions, t_q free]; one exp per j0 covers both
            # heads of the pair; PV contracts j=t_k with v as lhsT, producing
            # out^T [65, t_q] per head (row 64 = softmax denominators). Both
            # heads of a pair accumulate into one [P, 2, 512] PSUM tile.
            att = actsp.tile([P, 2, T], f16, tag="att", name=f"att{_rep}")

            def qk_exp_step(hp, i0, j0):
                kT = qkT[:, hp, :]
                qT = qkT[:, 2 + hp, :]
                c0 = P * j0 - 512 * i0
                c0p = max(0, c0)
                w = 512 - c0p
                psS = qkps.tile(
                    [P, 2, 512], f32, tag="qk", name=f"s{_rep}{hp}{i0}{j0}"
                )
                for h01 in range(2):
                    nc.tensor.matmul(
                        psS[:, h01, ds(c0p, w)],
                        kT[64 * h01 : 64 * h01 + 64, ts(j0, P)],
                        qT[64 * h01 : 64 * h01 + 64, ds(512 * i0 + c0p, w)],
                        start=True,
                        stop=True,
                    )
                pt = ptp.tile(
                    [P, 2, 512], f16, tag="pt", name=f"pt{_rep}{hp}{i0}{j0}"
                )
                nc.scalar.activation(
                    pt[:, :, ds(c0p, w)],
                    psS[:, :, ds(c0p, w)],
                    AF.Exp,
                    scale=SCALE,
                )
                if c0 >= 0:
                    for h01 in range(2):
                        nc.gpsimd.affine_select(
                            out=pt[:, h01, ds(c0, P)],
                            in_=pt[:, h01, ds(c0, P)],
                            compare_op=mybir.AluOpType.is_ge,
                            fill=0.0,
                            base=0,
                            pattern=[[1, P]],
                            channel_multiplier=-1,
                        )
                return pt

            def pv_step(hp, i0, j0, nj, acc, pt):
                cp = max(0, P * j0 - 512 * i0)
                wp = 512 - cp
                for h01 in range(2):
                    nc.tensor.matmul(
                        acc[0:65, h01, ds(cp, wp)],
                        v_all[:, j0, 2 * hp + h01, :],
                        pt[:, h01, ds(cp, wp)],
                        start=(j0 == 0),
                        stop=(j0 == nj - 1),
                    )

            def drain_block(hp, i0, acc):
                # Copy numerators out of PSUM (freeing the accumulator) and
                # compute fast reciprocals of the denominator rows.
                ac2 = ac2p.tile([P, 512], f32, tag="ac2", name=f"ac{_rep}{hp}{i0}")
                recs = []
                for h01 in range(2):
                    nc.vector.tensor_copy(
                        ac2[64 * h01 : 64 * h01 + 64, :], acc[0:64, h01, :]
                    )
                    dn = recp.tile(
                        [1, 512], f32, tag="dn", name=f"dn{_rep}{hp}{i0}{h01}"
                    )
                    nc.vector.tensor_copy(dn[:], acc[64:65, h01, :])
                    rec = recp.tile(
                        [1, 512], f32, tag="rec", name=f"rc{_rep}{hp}{i0}{h01}"
                    )
                    nc.vector.reciprocal_approx_fast(rec[:], dn[:])
                    recs.append(rec)
                return ac2, recs

            def normalize_i0(i0, drained):
                # att[64*h01 .. , hp, block] = numerators * (1/denominator)
                for hp in range(2):
                    ac2, recs = drained[hp]
                    for h01 in range(2):
                        rb = rbp.tile(
                            [P, 512], f32, tag="rb", name=f"rb{_rep}{hp}{i0}{h01}"
                        )
                        nc.gpsimd.partition_broadcast(rb[:], recs[h01][:])
                        sl = slice(64 * h01, 64 * h01 + 64)
                        nc.vector.tensor_mul(
                            att[sl, hp, ds(512 * i0, 512)],
                            ac2[sl, :],
                            rb[sl, :],
                        )

            steps = []
            for i0 in range(NI):
                for hp in range(2):
                    nj = 4 * i0 + 4
                    for j0 in range(nj):
                        steps.append((hp, i0, j0, nj))
            LAG = 2
            accs_map = {}
            pts_map = {}
            drained = {}
            y_after = {}
            for p in range(len(steps) + LAG):
                if p < len(steps):
                    hp, i0, j0, nj = steps[p]
                    if j0 == 0:
                        accs_map[(hp, i0)] = pvps.tile(
                            [P, 2, 512], f32, tag="pv", name=f"acc{_rep}{hp}{i0}"
                        )
                    pts_map[p] = qk_exp_step(hp, i0, j0)
                if p >= LAG:
                    hp, i0, j0, nj = steps[p - LAG]
                    pv_step(hp, i0, j0, nj, accs_map[(hp, i0)], pts_map.pop(p - LAG))
                    if j0 == nj - 1:
                        drained[(hp, i0)] = drain_block(hp, i0, accs_map.pop((hp, i0)))
                        if hp == 1:
                            normalize_i0(i0, (drained.pop((0, i0)), drained.pop((1, i0))))
                            y_after[p + 6] = i0
                if p in y_after:
                    i0y = y_after.pop(p)
                    for tt in range(4 * i0y, 4 * i0y + 4):
                        emit_y(tt)
            for i0y in sorted(y_after.values()):
                for tt in range(4 * i0y, 4 * i0y + 4):
                    emit_y(tt)

    nc.compile()
    return nc


def _get_nc(repeats=1):
    key = ("nc", repeats)
    if key not in _NC_CACHE:
        _NC_CACHE[key] = _build_nc(repeats)
    return _NC_CACHE[key]


def _make_in_maps(x, attn_w, attn_b, proj_w, proj_b):
    _make_in_maps.beff = {}
    in_maps = []
    for core in range(8):
        b, hg = core // 4, core % 4
        cs = 256 * hg
        k_cols = attn_w[:, cs : cs + 256]
        q_cols = attn_w[:, 1024 + cs : 1024 + cs + 256]
        v_cols = attn_w[:, 2048 + cs : 2048 + cs + 256]
        b_q = attn_b[1024 + cs : 1024 + cs + 256]
        b_v = attn_b[2048 + cs : 2048 + cs + 256]
        pw = proj_w[cs : cs + 256, :]
        beff = (b_v.astype(np.float64) @ pw.astype(np.float64)).astype(np.float32)
        if hg == 0:
            beff = beff + proj_b
        _make_in_maps.beff[core] = beff
        in_maps.append(
            {
                "xT": np.ascontiguousarray(x[b].T.astype(np.float16)),
                "wqk": np.ascontiguousarray(
                    np.concatenate([k_cols, q_cols], axis=1).astype(np.float16)
                ),
                "wv": np.ascontiguousarray(v_cols.astype(np.float16)),
                "bq": np.ascontiguousarray(b_q),
                "pw": np.ascontiguousarray(pw.astype(np.float16)),
            }
        )
    return in_maps


def kernel(x, attn_w, attn_b, proj_w, proj_b, _spmd_kwargs=None):
    from concourse.bass_utils import run_bass_kernel_spmd

    x = np.asarray(x, dtype=np.float32)
    attn_w = np.asarray(attn_w, dtype=np.float32)
    attn_b = np.asarray(attn_b, dtype=np.float32)
    proj_w = np.asarray(proj_w, dtype=np.float32)
    proj_b = np.asarray(proj_b, dtype=np.float32)

    nc = _get_nc((_spmd_kwargs or {}).pop("repeats", 1) if _spmd_kwargs else 1)
    in_maps = _make_in_maps(x, attn_w, attn_b, proj_w, proj_b)
    res = run_bass_kernel_spmd(
        nc, in_maps, core_ids=list(range(8)), **(_spmd_kwargs or {})
    )
    out = np.zeros((B, T, C), dtype=np.float32)
    for core in range(8):
        out[core // 4] += res.results[core]["y"].astype(np.float32)
    for core in range(8):
        out[core // 4] += _make_in_maps.beff[core][None, :]
    if _spmd_kwargs:
        kernel.last_results = res
    return out
